# revision 1
# baseline (speedup 1.0000x reference)
"""Causal self-attention on 8 Trainium2 NeuronCores (Bass/Tile).

Problem shape (hardcoded): x [2, 2048, 1024], W_attn [1024, 3072],
b_attn [3072], W_proj [1024, 1024], b_proj [1024], 16 heads, hd=64.

Sharding: tensor-parallel over (batch, head-group). Core k handles
batch k//4 and heads 4*(k%4) .. 4*(k%4)+3 (two head-pairs). Each core
computes its 4 heads' attention and a partial output projection
(y_local @ W_proj[rows]) of shape [2048, 1024]; the host sums the four
partials per batch and adds b_proj.
"""

import sys

for _p in ("/opt/trn_rl_repo", "/root/.axon_site/_ro/trn_rl_repo"):
    if _p not in sys.path:
        sys.path.insert(0, _p)

import numpy as np

import concourse.bass as bass  # noqa: F401  (engine types)
import concourse.mybir as mybir
import concourse.tile as tile
from concourse import bacc
from concourse.bass_utils import run_bass_kernel_spmd

F32 = mybir.dt.float32
F32R = mybir.dt.float32r

B = 2
T = 2048
C = 1024
H = 16
HD = 64
NCORES = 8
HEADS_PER_CORE = 4  # two pairs
PAIRS = 2
NKT = T // 128       # 16 k-tiles per head
NST = T // 512       # 4 q-strips per head
CKT = C // 128       # 8 contraction tiles for C

_CACHE = {}


def _build(phases=(1, 2, 3)):
    """Build the SPMD Bass program (identical for all cores)."""
    nc = bacc.Bacc(None, target_bir_lowering=False)

    xt_d = nc.dram_tensor("xt", [C, T], F32R, kind="ExternalInput")
    wq_d = nc.dram_tensor("wq", [128, PAIRS, CKT, 128], F32R, kind="ExternalInput")
    wk_d = nc.dram_tensor("wk", [128, PAIRS, CKT, 128], F32R, kind="ExternalInput")
    wv_d = nc.dram_tensor("wv", [128, PAIRS, CKT, 128], F32R, kind="ExternalInput")
    wp_d = nc.dram_tensor("wp", [128, 2, C], F32R, kind="ExternalInput")
    id_d = nc.dram_tensor("ident", [128, 128], F32R, kind="ExternalInput")
    ones_d = nc.dram_tensor("ones", [128, 1], F32R, kind="ExternalInput")
    mask_d = nc.dram_tensor("mask", [128, 128], F32R, kind="ExternalInput")
    out_d = nc.dram_tensor("out", [T, C], F32, kind="ExternalOutput")

    with tile.TileContext(nc) as tc, (
        tc.tile_pool(name="const", bufs=1)
    ) as const, (
        tc.tile_pool(name="weights", bufs=1)
    ) as wpool, (
        tc.tile_pool(name="acts", bufs=1)
    ) as apool, (
        tc.tile_pool(name="xstream", bufs=16)
    ) as xpool, (
        tc.tile_pool(name="ptp", bufs=3)
    ) as ppool, (
        tc.tile_pool(name="evict", bufs=3)
    ) as epool, (
        tc.tile_pool(name="dram_bounce", bufs=1, space="DRAM")
    ) as dpool:
        with (
            tc.tile_pool(name="st_ps", bufs=1, space="PSUM") as st_ps,
            tc.tile_pool(name="y_ps", bufs=1, space="PSUM") as y_ps,
            tc.tile_pool(name="qkv_ps", bufs=1, space="PSUM") as qkv_ps,
        ):
            ident = const.tile([128, 128], F32R)
            mask_tri = const.tile([128, 128], F32R)
            nc.sync.dma_start(ident[:], id_d[:])
            nc.sync.dma_start(mask_tri[:], mask_d[:])

            wq = wpool.tile([128, PAIRS, CKT, 128], F32R)
            wk = wpool.tile([128, PAIRS, CKT, 128], F32R)
            wv = wpool.tile([128, PAIRS, CKT, 128], F32R)
            wp = wpool.tile([128, 2, C], F32R)
            # pair-0 weight loads first so its matmuls start early; pair-1
            # loads are issued after the first x strip (wp just before proj)
            nc.sync.dma_start(wq[:, 0], wq_d[:, 0])
            nc.sync.dma_start(wk[:, 0], wk_d[:, 0])
            nc.sync.dma_start(wv[:, 0], wv_d[:, 0])

            # activations kept resident in SBUF
            qt = apool.tile([128, PAIRS, T], F32R)   # q^T, heads stacked in pairs
            kt = apool.tile([128, PAIRS, T], F32R)   # k^T
            # v^T is dead after the phase-1b transposes; ytn is written only in
            # phase 2b — share one buffer (Tile serializes the WAR hazard).
            vt = apool.tile([128, PAIRS, T], F32R)   # v^T (pre-transpose)
            ytn = vt
            v_nat = [
                apool.tile([128, NKT, HD + 1], F32R, name=f"vnat{i}", tag=f"vnat{i}")
                for i in range(HEADS_PER_CORE)
            ]
            ytu = apool.tile([128, PAIRS, T], F32R)  # unnormalized y^T
            sums_dram = dpool.tile([4, NST, 512], F32)

            # ---- Phase 1: QKV projections (transposed outputs) ----
            # loop q-strips of T; stream x^T chunks [128, 512]
            def emit_qkv(s):
                xch = [None] * CKT
                for kc in range(CKT):
                    xc = xpool.tile([128, 512], F32R, name=f"xc_{s}_{kc}", tag="xc")
                    nc.sync.dma_start(xc[:], xt_d[kc * 128:(kc + 1) * 128, s * 512:(s + 1) * 512])
                    xch[kc] = xc
                if s == 0:
                    nc.sync.dma_start(wq[:, 1], wq_d[:, 1])
                    nc.sync.dma_start(wk[:, 1], wk_d[:, 1])
                    nc.sync.dma_start(wv[:, 1], wv_d[:, 1])
                for p in range(PAIRS):
                    for w_t, dest in ((wq, qt), (wk, kt), (wv, vt)):
                        ps = qkv_ps.tile([128, 512], F32, name=f"qkvps_{s}_{p}", tag=f"qkv{p}")
                        for kc in range(CKT):
                            nc.tensor.matmul(
                                ps[:],
                                w_t[:, p, kc, :],
                                xch[kc][:],
                                start=(kc == 0),
                                stop=(kc == CKT - 1),
                            )
                        nc.vector.tensor_copy(dest[:, p, s * 512:(s + 1) * 512], ps[:])
                    # v natural layout for this strip via PE transpose:
                    # 4 t-tiles per psum slot, one wide eviction
                    for h in range(2):
                        hh = 2 * p + h
                        pt = qkv_ps.tile([128, 4 * HD], F32R, name=f"vtp_{s}_{p}_{h}", tag=f"qkv{p}")
                        for i, t in enumerate(range(4 * s, 4 * s + 4)):
                            nc.tensor.transpose(
                                pt[:, i * HD:(i + 1) * HD],
                                vt[h * HD:(h + 1) * HD, p, t * 128:(t + 1) * 128],
                                ident[h * HD:(h + 1) * HD, h * HD:(h + 1) * HD],
                            )
                        nc.vector.tensor_copy(
                            v_nat[hh][:, 4 * s:4 * s + 4, 0:HD],
                            pt[:].rearrange("p (t d) -> p t d", t=4),
                        )

            for hh in range(HEADS_PER_CORE):
                nc.sync.dma_start(
                    v_nat[hh][:, :, HD:HD + 1],
                    ones_d[:].to_broadcast((128, NKT, 1)),
                )

            # ---- Phase 2: attention ----
            recip_dram = dpool.tile([4, NST, 512], F32)

            def emit_attn(s):
                n_k = 4 * s + 4  # k-tiles for this strip (causal)
                for p in range(PAIRS):
                    ytile = [
                        y_ps.tile([HD + 1, 512], F32, name=f"y_{p}_{s}_{h}", tag=f"y{h}")
                        for h in range(2)
                    ]
                    ngrp = (n_k + 1) // 2
                    for g in range(ngrp):
                        st = [
                            st_ps.tile([128, 1024], F32, name=f"st_{p}_{s}_{g}_{h}", tag="st", bufs=2)
                            for h in range(2)
                        ]
                        njj = min(2, n_k - 2 * g)
                        # S^T matmuls, interleaved across heads for row-group overlap
                        for jj in range(njj):
                            j = 2 * g + jj
                            c0 = max(0, 128 * (j - 4 * s))
                            for h in range(2):
                                nc.tensor.matmul(
                                    st[h][:, jj * 512 + c0:(jj + 1) * 512],
                                    kt[h * HD:(h + 1) * HD, p, j * 128:(j + 1) * 128],
                                    qt[h * HD:(h + 1) * HD, p, s * 512 + c0:(s + 1) * 512],
                                    start=True,
                                    stop=True,
                                )
                        # exp (full group; unwritten cols are never read downstream)
                        for h in range(2):
                            hh = 2 * p + h
                            ptile = ppool.tile([128, 1024], F32R, name=f"pt_{p}_{s}_{g}_{h}", tag="pt", bufs=5)
                            nc.scalar.activation(
                                ptile[:], st[h][:], mybir.ActivationFunctionType.Exp
                            )
                            for jj in range(njj):
                                j = 2 * g + jj
                                c0 = max(0, 128 * (j - 4 * s))
                                if j >= 4 * s:  # diagonal block: triangular mask
                                    blk = ptile[:, jj * 512 + c0:jj * 512 + c0 + 128]
                                    nc.gpsimd.tensor_mul(blk, blk, mask_tri[:])
                                nc.tensor.matmul(
                                    ytile[h][:, c0:512],
                                    v_nat[hh][:, j, :],
                                    ptile[:, jj * 512 + c0:(jj + 1) * 512],
                                    start=(j == 0),
                                    stop=(j == n_k - 1),
                                )
                    # evict y (rows 0:64) and sums (row 64)
                    for h in range(2):
                        r = 2 * p + h
                        nc.vector.tensor_copy(
                            ytu[h * HD:(h + 1) * HD, p, s * 512:(s + 1) * 512],
                            ytile[h][0:HD, :],
                        )
                        srow = ppool.tile([HD + 1, 512], F32, name=f"srow_{s}_{r}", tag="srow")
                        nc.vector.tensor_copy(srow[HD:HD + 1, :], ytile[h][HD:HD + 1, :])
                        nc.sync.dma_start(sums_dram[r:r + 1, s, :], srow[HD:HD + 1, :])

                # ---- per-strip normalization ----
                sums_s = ppool.tile([4, 512], F32, name=f"sums_{s}", tag="sums")
                recip_s = ppool.tile([4, 512], F32, name=f"recip_{s}", tag="recip")
                rscr_s = ppool.tile([4, 512], F32, name=f"rscr_{s}", tag="rscr")
                nc.sync.dma_start(sums_s[:], sums_dram[:, s, :])
                nc.vector.reciprocal_approx_accurate(recip_s[:], sums_s[:], rscr_s[:])
                nc.sync.dma_start(recip_dram[:, s, :], recip_s[:])
                for p in range(PAIRS):
                    for h in range(2):
                        r = 2 * p + h
                        rb = ppool.tile([128, 512], F32, name=f"rb_{s}_{r}", tag="rb")
                        nc.sync.dma_start(
                            rb[h * HD:(h + 1) * HD, :],
                            recip_dram[r:r + 1, s, :].to_broadcast((HD, 512)),
                        )
                        nc.vector.tensor_mul(
                            ytn[h * HD:(h + 1) * HD, p, s * 512:(s + 1) * 512],
                            ytu[h * HD:(h + 1) * HD, p, s * 512:(s + 1) * 512],
                            rb[h * HD:(h + 1) * HD, :].bitcast(F32R),
                        )

            # ---- skewed software pipeline: attention trails QKV by 1 strip ----
            if 1 in phases:
                emit_qkv(0)
            for s in range(NST) if 2 in phases else []:
                if s + 1 < NST and 1 in phases:
                    emit_qkv(s + 1)
                emit_attn(s)

        # ---- Phase 3: output projection (partial) ----
        with tc.tile_pool(name="o_ps", bufs=2, space="PSUM") as o_ps:
            for f in range(2):
                nc.sync.dma_start(wp[:, f, :], wp_d[:, f, :])
            for t in range(NKT) if 3 in phases else []:
                op = o_ps.tile([128, 1024], F32, name=f"op_{t}", tag="op")
                for f in range(2):
                    for n in range(2):
                        nc.tensor.matmul(
                            op[:, n * 512:(n + 1) * 512],
                            ytn[:, f, t * 128:(t + 1) * 128],
                            wp[:, f, n * 512:(n + 1) * 512],
                            start=(f == 0),
                            stop=(f == 1),
                        )
                ot = epool.tile([128, 1024], F32, name=f"ot_{t}", tag="ot")
                nc.vector.tensor_copy(ot[:], op[:])
                nc.sync.dma_start(out_d[t * 128:(t + 1) * 128, :], ot[:])

    nc.compile()
    return nc


def _prep_inputs(x, W_attn, b_attn, W_proj):
    """Per-core input maps. Core k: batch k//4, head-group k%4."""
    assert np.allclose(b_attn, 0.0), "nonzero b_attn not supported by this kernel"
    scale = 1.0 / np.sqrt(np.float32(HD))

    ident = np.eye(128, dtype=np.float32)
    ones = np.ones((128, 1), dtype=np.float32)
    mask = (np.arange(128)[:, None] <= np.arange(128)[None, :]).astype(np.float32)

    def lhsT_tiles(w):
        # [C, 128] -> [128, CKT, 128] with [p, t, c] = w[t*128+p, c]
        return np.ascontiguousarray(w.reshape(CKT, 128, 128).transpose(1, 0, 2))

    in_maps = []
    for core in range(NCORES):
        b = core // 4
        g = core % 4
        heads = [4 * g + i for i in range(HEADS_PER_CORE)]
        xt = np.ascontiguousarray(x[b].T)  # [C, T]

        def w_slice(base, hs, sc=1.0):
            cols = np.concatenate(
                [np.arange(base + h * HD, base + (h + 1) * HD) for h in hs]
            )
            return np.ascontiguousarray(W_attn[:, cols]) * sc

        wq = np.stack(
            [lhsT_tiles(w_slice(0, heads[2 * p:2 * p + 2], scale)) for p in range(PAIRS)], axis=1
        )  # [128, PAIRS, CKT, 128]
        wk = np.stack(
            [lhsT_tiles(w_slice(C, heads[2 * p:2 * p + 2])) for p in range(PAIRS)], axis=1
        )
        wv = np.stack(
            [lhsT_tiles(w_slice(2 * C, heads[2 * p:2 * p + 2])) for p in range(PAIRS)], axis=1
        )
        # W_proj rows for this head group: [256, C] -> [128, 2, C]
        wp_rows = W_proj[heads[0] * HD:(heads[-1] + 1) * HD, :]
        wp = np.ascontiguousarray(wp_rows.reshape(2, 128, C).transpose(1, 0, 2))

        in_maps.append(
            {
                "xt": np.ascontiguousarray(xt, dtype=np.float32),
                "wq": np.ascontiguousarray(wq, dtype=np.float32),
                "wk": np.ascontiguousarray(wk, dtype=np.float32),
                "wv": np.ascontiguousarray(wv, dtype=np.float32),
                "wp": np.ascontiguousarray(wp, dtype=np.float32),
                "ident": ident,
                "ones": ones,
                "mask": mask,
            }
        )
    return in_maps


def kernel(x, W_attn, b_attn, W_proj, b_proj, _want_results=False, _spmd_kwargs=None):
    x = np.asarray(x, dtype=np.float32)
    W_attn = np.asarray(W_attn, dtype=np.float32)
    b_attn = np.asarray(b_attn, dtype=np.float32)
    W_proj = np.asarray(W_proj, dtype=np.float32)
    b_proj = np.asarray(b_proj, dtype=np.float32)

    if "nc" not in _CACHE:
        _CACHE["nc"] = _build()
    nc = _CACHE["nc"]

    in_maps = _prep_inputs(x, W_attn, b_attn, W_proj)
    kw = dict(_spmd_kwargs or {})
    res = run_bass_kernel_spmd(nc, in_maps, list(range(NCORES)), **kw)

    out = np.zeros((B, T, C), dtype=np.float32)
    for core in range(NCORES):
        out[core // 4] += res.results[core]["out"]
    out += b_proj[None, None, :]
    if _want_results:
        return out, res
    return out



# revision 13
# speedup vs baseline: 1.1241x; 1.1241x over previous
"""Causal self-attention on 8 Trainium2 NeuronCores (Bass/Tile).

Problem shape (hardcoded): x [2, 2048, 1024], W_attn [1024, 3072],
b_attn [3072], W_proj [1024, 1024], b_proj [1024], 16 heads, hd=64.

Sharding: tensor-parallel over (batch, head-group). Core k handles
batch k//4 and heads 4*(k%4) .. 4*(k%4)+3 (two head-pairs). Each core
computes its 4 heads' attention and a partial output projection
(y_local @ W_proj[rows]) of shape [2048, 1024]; the host sums the four
partials per batch and adds b_proj.

v3: bf16 activations/weights (fp32 PSUM accumulation), V produced
directly in natural [t, d] layout (no PE transposes), Q/K PSUM
evictions on the scalar engine, per-pair softmax normalization with
the reciprocal read straight out of the y-eviction tile's sums row,
batched per-strip x DMAs, and attention emitted at higher priority
than the next strip's QKV and the previous strip's projection so the
tensor engine fills its softmax stalls with those matmuls.
"""

import sys

for _p in ("/opt/trn_rl_repo", "/root/.axon_site/_ro/trn_rl_repo"):
    if _p not in sys.path:
        sys.path.insert(0, _p)

import ml_dtypes
import numpy as np

import concourse.bass as bass  # noqa: F401  (engine types)
import concourse.mybir as mybir
import concourse.tile as tile
from concourse import bacc
from concourse.bass_utils import run_bass_kernel_spmd

F32 = mybir.dt.float32
F32R = mybir.dt.float32r
BF16 = mybir.dt.bfloat16
NP_BF16 = ml_dtypes.bfloat16

B = 2
T = 2048
C = 1024
H = 16
HD = 64
NCORES = 8
HEADS_PER_CORE = 4  # two pairs
PAIRS = 2
NKT = T // 128       # 16 k-tiles per head
NST = T // 512       # 4 q-strips per head
CKT = C // 128       # 8 contraction tiles for C

_CACHE = {}


def _build():
    """Build the SPMD Bass program (identical for all cores)."""
    nc = bacc.Bacc(None, target_bir_lowering=False)

    # x^T pre-tiled on host: [p, kc, t] = x^T[kc*128+p, t]
    xt_d = nc.dram_tensor("xt", [128, CKT, T], BF16, kind="ExternalInput")
    wq_d = nc.dram_tensor("wq", [128, PAIRS, CKT, 128], BF16, kind="ExternalInput")
    wk_d = nc.dram_tensor("wk", [128, PAIRS, CKT, 128], BF16, kind="ExternalInput")
    wv_d = nc.dram_tensor("wv", [128, CKT, 256], BF16, kind="ExternalInput")
    wp_d = nc.dram_tensor("wp", [128, 2, C], F32R, kind="ExternalInput")
    mask_d = nc.dram_tensor("mask", [128, 128], BF16, kind="ExternalInput")
    out_d = nc.dram_tensor("out", [T, C], F32, kind="ExternalOutput")

    with tile.TileContext(nc) as tc, (
        tc.tile_pool(name="const", bufs=1)
    ) as const, (
        tc.tile_pool(name="weights", bufs=1)
    ) as wpool, (
        tc.tile_pool(name="acts", bufs=1)
    ) as apool, (
        tc.tile_pool(name="xstream", bufs=3)
    ) as xpool, (
        tc.tile_pool(name="ptp", bufs=3)
    ) as ppool, (
        tc.tile_pool(name="evict", bufs=3)
    ) as epool, (
        tc.tile_pool(name="dram_bounce", bufs=1, space="DRAM")
    ) as dpool, (
        tc.tile_pool(name="st_ps", bufs=1, space="PSUM")
    ) as st_ps, (
        tc.tile_pool(name="y_ps", bufs=1, space="PSUM")
    ) as y_ps, (
        tc.tile_pool(name="qkv_ps", bufs=1, space="PSUM")
    ) as qkv_ps:
        mask_tri = const.tile([128, 128], BF16)

        wq = wpool.tile([128, PAIRS, CKT, 128], BF16)
        wk = wpool.tile([128, PAIRS, CKT, 128], BF16)
        wv = wpool.tile([128, CKT, 256], BF16)
        wp = wpool.tile([128, 2, C], F32R)

        # activations kept resident in SBUF
        qt = apool.tile([128, PAIRS, T], BF16)   # q^T, heads stacked in pairs
        kt = apool.tile([128, PAIRS, T], BF16)   # k^T
        v_nat = apool.tile([128, NKT, HEADS_PER_CORE, HD + 1], BF16)
        ytn = apool.tile([128, PAIRS, T], F32R)  # normalized y^T
        rscr = apool.tile([1, PAIRS, 512], F32)  # reciprocal scratch

        # ---- lead-in DMAs: first strip of x + pair-0 weights first ----
        xs0 = xpool.tile([128, CKT, 512], BF16, name="xc_0", tag="xc")
        nc.sync.dma_start(xs0[:, 0:4], xt_d[:, 0:4, 0:512])
        nc.sync.dma_start(wq[:, 0], wq_d[:, 0])
        nc.sync.dma_start(xs0[:, 4:8], xt_d[:, 4:8, 0:512])
        nc.sync.dma_start(wk[:, 0], wk_d[:, 0])
        nc.sync.dma_start(wv[:], wv_d[:])
        nc.sync.dma_start(mask_tri[:], mask_d[:])
        nc.sync.dma_start(wq[:, 1], wq_d[:, 1])
        nc.sync.dma_start(wk[:, 1], wk_d[:, 1])
        nc.sync.dma_start(wp[:], wp_d[:])
        # ones column of v_nat (the 65th lhsT column yields softmax sums)
        for hh in range(HEADS_PER_CORE):
            nc.gpsimd.memset(v_nat[:, :, hh, HD:HD + 1], 1.0)

        # ---- QKV (q^T/k^T transposed; v natural) ----
        def emit_qkv(s, xs):
            if xs is None:
                xs = xpool.tile([128, CKT, 512], BF16, name=f"xc_{s}", tag="xc")
                nc.sync.dma_start(xs[:], xt_d[:, :, s * 512:(s + 1) * 512])
            for p in range(PAIRS):
                for w_t, dest in ((wq, qt), (wk, kt)):
                    ps = qkv_ps.tile([128, 512], F32, name=f"qkps_{s}_{p}_{0 if w_t is wq else 1}", tag=f"qkv{p}")
                    for kc in range(CKT):
                        nc.tensor.matmul(
                            ps[:],
                            w_t[:, p, kc, :],
                            xs[:, kc, :],
                            start=(kc == 0),
                            stop=(kc == CKT - 1),
                        )
                    # evict on the scalar engine (Act is idle during QKV)
                    nc.scalar.copy(dest[:, p, s * 512:(s + 1) * 512], ps[:])
            # v in natural layout: x^T chunks as lhsT
            for i in range(4):
                t = 4 * s + i
                psv = qkv_ps.tile([128, 256], F32, name=f"vps_{s}_{i}", tag=f"qkv{i % 2}")
                for kc in range(CKT):
                    nc.tensor.matmul(
                        psv[:],
                        xs[:, kc, i * 128:(i + 1) * 128],
                        wv[:, kc, :],
                        start=(kc == 0),
                        stop=(kc == CKT - 1),
                    )
                nc.vector.tensor_copy(
                    v_nat[:, t, :, 0:HD],
                    psv[:].rearrange("p (h d) -> p h d", h=HEADS_PER_CORE),
                )

        # ---- attention ----
        recip_dram = dpool.tile([NST, 4, 512], F32)
        sums_dram = dpool.tile([NST, 4, 512], F32)

        def emit_attn(s):
            n_k = 4 * s + 4  # k-tiles for this strip (causal)
            yu_all = ppool.tile([HD + 1, HEADS_PER_CORE, 512], F32, name=f"yu_{s}", tag="yu", bufs=2)
            for p in range(PAIRS):
                ytile = [
                    y_ps.tile([HD + 1, 512], F32, name=f"y_{p}_{s}_{h}", tag=f"y{h}")
                    for h in range(2)
                ]
                ngrp = n_k // 2
                for g in range(ngrp):
                    st = [
                        st_ps.tile([128, 1024], F32, name=f"st_{p}_{s}_{g}_{h}", tag="st", bufs=2)
                        for h in range(2)
                    ]
                    # S^T matmuls, interleaved across heads for row-group overlap
                    for jj in range(2):
                        j = 2 * g + jj
                        c0 = max(0, 128 * (j - 4 * s))
                        for h in range(2):
                            nc.tensor.matmul(
                                st[h][:, jj * 512 + c0:(jj + 1) * 512],
                                kt[h * HD:(h + 1) * HD, p, j * 128:(j + 1) * 128],
                                qt[h * HD:(h + 1) * HD, p, s * 512 + c0:(s + 1) * 512],
                                start=True,
                                stop=True,
                            )
                    # exp, cropped to the first valid column of the group
                    ec0 = max(0, 128 * (2 * g - 4 * s))
                    for h in range(2):
                        hh = 2 * p + h
                        ptile = ppool.tile([128, 1024], BF16, name=f"pt_{p}_{s}_{g}_{h}", tag="pt", bufs=5)
                        nc.scalar.activation(
                            ptile[:, ec0:1024], st[h][:, ec0:1024],
                            mybir.ActivationFunctionType.Exp,
                        )
                        for jj in range(2):
                            j = 2 * g + jj
                            c0 = max(0, 128 * (j - 4 * s))
                            if j >= 4 * s:  # diagonal block: triangular mask
                                blk = ptile[:, jj * 512 + c0:jj * 512 + c0 + 128]
                                nc.gpsimd.tensor_mul(blk, blk, mask_tri[:])
                            nc.tensor.matmul(
                                ytile[h][:, c0:512],
                                v_nat[:, j, hh, :],
                                ptile[:, jj * 512 + c0:(jj + 1) * 512],
                                start=(j == 0),
                                stop=(j == n_k - 1),
                            )
                # evict y + sums rows to SBUF, freeing the PSUM banks
                for h in range(2):
                    hh = 2 * p + h
                    nc.vector.tensor_copy(yu_all[:, hh, :], ytile[h][:])

            # ---- per-strip normalization: 2-hop DRAM gather/broadcast ----
            sums4 = ppool.tile([4, 512], F32, name=f"sums_{s}", tag="sums", bufs=2)
            recip4 = ppool.tile([4, 512], F32, name=f"recip_{s}", tag="recip", bufs=2)
            rscr4 = ppool.tile([4, 512], F32, name=f"rscr_{s}", tag="rscr", bufs=2)
            rb_all = ppool.tile([HD, HEADS_PER_CORE, 512], F32, name=f"rb_{s}", tag="rb", bufs=2)
            nc.sync.dma_start(sums_dram[s], yu_all[HD:HD + 1, :, :])
            nc.sync.dma_start(sums4[:], sums_dram[s])
            nc.vector.reciprocal_approx_accurate(recip4[:], sums4[:], rscr4[:])
            nc.sync.dma_start(recip_dram[s], recip4[:])
            nc.sync.dma_start(
                rb_all[:], recip_dram[s:s + 1].to_broadcast((HD, 4, 512))
            )
            for p in range(PAIRS):
                for h in range(2):
                    r = 2 * p + h
                    nc.gpsimd.tensor_mul(
                        ytn[h * HD:(h + 1) * HD, p, s * 512:(s + 1) * 512],
                        yu_all[0:HD, r, :],
                        rb_all[:, r, :],
                    )

        # ---- output projection (partial), interleaved per strip ----
        def emit_proj(s):
            for ti in range(4):
                t = 4 * s + ti
                ot = epool.tile([128, 1024], F32, name=f"ot_{t}", tag="ot", bufs=3)
                for n in range(2):
                    op = qkv_ps.tile([128, 512], F32, name=f"op_{t}_{n}", tag=f"qkv{n}")
                    for f in range(2):
                        nc.tensor.matmul(
                            op[:],
                            ytn[:, f, t * 128:(t + 1) * 128],
                            wp[:, f, n * 512:(n + 1) * 512],
                            start=(f == 0),
                            stop=(f == 1),
                        )
                    nc.vector.tensor_copy(ot[:, n * 512:(n + 1) * 512], op[:])
                nc.sync.dma_start(out_d[t * 128:(t + 1) * 128, :], ot[:])

        # ---- skewed software pipeline ----
        # per slot: attention first (highest priority), next strip's QKV and
        # the previous strip's projection as PE stall-filler
        emit_qkv(0, xs0)
        for s in range(NST):
            emit_attn(s)
            if s + 1 < NST:
                emit_qkv(s + 1, None)
            if s >= 1:
                emit_proj(s - 1)
        emit_proj(NST - 1)

    nc.compile()
    return nc


def _prep_inputs(x, W_attn, b_attn, W_proj):
    """Per-core input maps. Core k: batch k//4, head-group k%4."""
    assert np.allclose(b_attn, 0.0), "nonzero b_attn not supported by this kernel"
    scale = 1.0 / np.sqrt(np.float32(HD))

    mask = (np.arange(128)[:, None] <= np.arange(128)[None, :]).astype(NP_BF16)

    def lhsT_tiles(w):
        # [C, 128] -> [128, CKT, 128] with [p, t, c] = w[t*128+p, c]
        return np.ascontiguousarray(w.reshape(CKT, 128, 128).transpose(1, 0, 2))

    in_maps = []
    for core in range(NCORES):
        b = core // 4
        g = core % 4
        heads = [4 * g + i for i in range(HEADS_PER_CORE)]
        # [128, CKT, T] with [p, kc, t] = x^T[kc*128+p, t]
        xt = np.ascontiguousarray(
            x[b].T.reshape(CKT, 128, T).transpose(1, 0, 2)
        ).astype(NP_BF16)

        def w_slice(base, hs, sc=1.0):
            cols = np.concatenate(
                [np.arange(base + h * HD, base + (h + 1) * HD) for h in hs]
            )
            return np.ascontiguousarray(W_attn[:, cols]) * sc

        wq = np.stack(
            [lhsT_tiles(w_slice(0, heads[2 * p:2 * p + 2], scale)) for p in range(PAIRS)], axis=1
        ).astype(NP_BF16)  # [128, PAIRS, CKT, 128]
        wk = np.stack(
            [lhsT_tiles(w_slice(C, heads[2 * p:2 * p + 2])) for p in range(PAIRS)], axis=1
        ).astype(NP_BF16)
        # v weights in natural rhs layout: [128, CKT, 256], [p, kc, d] = Wv[kc*128+p, d]
        wv_cols = w_slice(2 * C, heads)  # [C, 256]
        wv = np.ascontiguousarray(
            wv_cols.reshape(CKT, 128, 256).transpose(1, 0, 2)
        ).astype(NP_BF16)
        # W_proj rows for this head group: [256, C] -> [128, 2, C]
        wp_rows = W_proj[heads[0] * HD:(heads[-1] + 1) * HD, :]
        wp = np.ascontiguousarray(wp_rows.reshape(2, 128, C).transpose(1, 0, 2))

        in_maps.append(
            {
                "xt": np.ascontiguousarray(xt),
                "wq": np.ascontiguousarray(wq),
                "wk": np.ascontiguousarray(wk),
                "wv": np.ascontiguousarray(wv),
                "wp": np.ascontiguousarray(wp, dtype=np.float32),
                "mask": np.ascontiguousarray(mask),
            }
        )
    return in_maps


def kernel(x, W_attn, b_attn, W_proj, b_proj, _want_results=False, _spmd_kwargs=None):
    x = np.asarray(x, dtype=np.float32)
    W_attn = np.asarray(W_attn, dtype=np.float32)
    b_attn = np.asarray(b_attn, dtype=np.float32)
    W_proj = np.asarray(W_proj, dtype=np.float32)
    b_proj = np.asarray(b_proj, dtype=np.float32)

    if "nc" not in _CACHE:
        _CACHE["nc"] = _build()
    nc = _CACHE["nc"]

    in_maps = _prep_inputs(x, W_attn, b_attn, W_proj)
    kw = dict(_spmd_kwargs or {})
    res = run_bass_kernel_spmd(nc, in_maps, list(range(NCORES)), **kw)

    out = np.zeros((B, T, C), dtype=np.float32)
    for core in range(NCORES):
        out[core // 4] += np.asarray(res.results[core]["out"], dtype=np.float32)
    out += b_proj[None, None, :]
    if _want_results:
        return out, res
    return out


# revision 36
# speedup vs baseline: 1.1715x; 1.0421x over previous
"""Causal self-attention on 8 Trainium2 NeuronCores (Bass/Tile).

Problem shape (hardcoded): x [2, 2048, 1024], W_attn [1024, 3072],
b_attn [3072], W_proj [1024, 1024], b_proj [1024], 16 heads, hd=64.

Sharding: tensor-parallel over (batch, head-group). Core k handles
batch k//4 and heads 4*(k%4) .. 4*(k%4)+3 (two head-pairs). Each core
computes its 4 heads' attention and a partial output projection
(y_local @ W_proj[rows]) of shape [2048, 1024]; the host sums the four
partials per batch and adds b_proj.

v3: bf16 activations/weights (fp32 PSUM accumulation), V produced
directly in natural [t, d] layout (no PE transposes), Q/K PSUM
evictions on the scalar engine, per-pair softmax normalization with
the reciprocal read straight out of the y-eviction tile's sums row,
batched per-strip x DMAs, and attention emitted at higher priority
than the next strip's QKV and the previous strip's projection so the
tensor engine fills its softmax stalls with those matmuls.
"""

import sys

for _p in ("/opt/trn_rl_repo", "/root/.axon_site/_ro/trn_rl_repo"):
    if _p not in sys.path:
        sys.path.insert(0, _p)

import ml_dtypes
import numpy as np

import concourse.bass as bass  # noqa: F401  (engine types)
import concourse.mybir as mybir
import concourse.tile as tile
from concourse import bacc
from concourse.bass_utils import run_bass_kernel_spmd

F32 = mybir.dt.float32
F32R = mybir.dt.float32r
BF16 = mybir.dt.bfloat16
NP_BF16 = ml_dtypes.bfloat16

B = 2
T = 2048
C = 1024
H = 16
HD = 64
NCORES = 8
HEADS_PER_CORE = 4  # two pairs
PAIRS = 2
NKT = T // 128       # 16 k-tiles per head
NST = T // 512       # 4 q-strips per head
CKT = C // 128       # 8 contraction tiles for C

_CACHE = {}


def _build():
    """Build the SPMD Bass program (identical for all cores)."""
    nc = bacc.Bacc(None, target_bir_lowering=False)

    # x^T pre-tiled on host: [p, kc, t] = x^T[kc*128+p, t]
    xt_d = nc.dram_tensor("xt", [128, CKT, T], BF16, kind="ExternalInput")
    wq_d = nc.dram_tensor("wq", [128, PAIRS, CKT, 128], BF16, kind="ExternalInput")
    wk_d = nc.dram_tensor("wk", [128, PAIRS, CKT, 128], BF16, kind="ExternalInput")
    wv_d = nc.dram_tensor("wv", [128, CKT, 256], BF16, kind="ExternalInput")
    wp_d = nc.dram_tensor("wp", [128, 2, C], F32R, kind="ExternalInput")
    mask_d = nc.dram_tensor("mask", [128, 128], BF16, kind="ExternalInput")
    ones_d = nc.dram_tensor("ones", [128, 1], BF16, kind="ExternalInput")
    out_d = nc.dram_tensor("out", [T, C], F32, kind="ExternalOutput")

    with tile.TileContext(nc) as tc, (
        tc.tile_pool(name="const", bufs=1)
    ) as const, (
        tc.tile_pool(name="weights", bufs=1)
    ) as wpool, (
        tc.tile_pool(name="acts", bufs=1)
    ) as apool, (
        tc.tile_pool(name="xstream", bufs=3)
    ) as xpool, (
        tc.tile_pool(name="ptp", bufs=3)
    ) as ppool, (
        tc.tile_pool(name="evict", bufs=3)
    ) as epool, (
        tc.tile_pool(name="dram_bounce", bufs=1, space="DRAM")
    ) as dpool, (
        tc.tile_pool(name="st_ps", bufs=1, space="PSUM")
    ) as st_ps, (
        tc.tile_pool(name="y_ps", bufs=1, space="PSUM")
    ) as y_ps, (
        tc.tile_pool(name="qkv_ps", bufs=1, space="PSUM")
    ) as qkv_ps:
        mask_tri = const.tile([128, 128], BF16)

        wq = wpool.tile([128, PAIRS, CKT, 128], BF16)
        wk = wpool.tile([128, PAIRS, CKT, 128], BF16)
        wv = wpool.tile([128, CKT, 256], BF16)
        wp = wpool.tile([128, 2, C], F32R)

        # activations kept resident in SBUF
        qt = apool.tile([128, PAIRS, T], BF16)   # q^T, heads stacked in pairs
        kt = apool.tile([128, PAIRS, T], BF16)   # k^T
        v_nat = apool.tile([128, NKT, HEADS_PER_CORE, HD + 1], BF16)
        ytn = apool.tile([128, PAIRS, T], F32R)  # normalized y^T

        # ---- lead-in DMAs: first strip of x + pair-0 weights first ----
        xs0 = xpool.tile([128, CKT, 512], BF16, name="xc_0", tag="xc")
        nc.sync.dma_start(xs0[:, 0:4], xt_d[:, 0:4, 0:512])
        nc.sync.dma_start(wq[:, 0], wq_d[:, 0])
        nc.sync.dma_start(xs0[:, 4:8], xt_d[:, 4:8, 0:512])
        nc.sync.dma_start(wk[:, 0], wk_d[:, 0])
        nc.sync.dma_start(wv[:], wv_d[:])
        nc.sync.dma_start(mask_tri[:], mask_d[:])
        nc.sync.dma_start(wq[:, 1], wq_d[:, 1])
        nc.sync.dma_start(wk[:, 1], wk_d[:, 1])
        nc.sync.dma_start(wp[:], wp_d[:])
        # ones column of v_nat (the 65th lhsT column yields softmax sums)
        for hh in range(HEADS_PER_CORE):
            nc.sync.dma_start(
                v_nat[:, :, hh, HD:HD + 1], ones_d[:].to_broadcast((128, NKT, 1))
            )

        # ---- QKV (q^T/k^T transposed; v natural) ----
        def emit_qkv(s, xs):
            if xs is None:
                xs = xpool.tile([128, CKT, 512], BF16, name=f"xc_{s}", tag="xc")
                nc.sync.dma_start(xs[:], xt_d[:, :, s * 512:(s + 1) * 512])
            for p in range(PAIRS):
                for w_t, dest in ((wq, qt), (wk, kt)):
                    ps = qkv_ps.tile([128, 512], F32, name=f"qkps_{s}_{p}_{0 if w_t is wq else 1}", tag=f"qkv{p}")
                    for kc in range(CKT):
                        nc.tensor.matmul(
                            ps[:],
                            w_t[:, p, kc, :],
                            xs[:, kc, :],
                            start=(kc == 0),
                            stop=(kc == CKT - 1),
                        )
                    # evict on the scalar engine (Act is idle during QKV)
                    nc.scalar.copy(dest[:, p, s * 512:(s + 1) * 512], ps[:])
            # v in natural layout: x^T chunks as lhsT
            for i in range(4):
                t = 4 * s + i
                psv = qkv_ps.tile([128, 256], F32, name=f"vps_{s}_{i}", tag=f"qkv{i % 2}")
                for kc in range(CKT):
                    nc.tensor.matmul(
                        psv[:],
                        xs[:, kc, i * 128:(i + 1) * 128],
                        wv[:, kc, :],
                        start=(kc == 0),
                        stop=(kc == CKT - 1),
                    )
                nc.vector.tensor_copy(
                    v_nat[:, t, :, 0:HD],
                    psv[:].rearrange("p (h d) -> p h d", h=HEADS_PER_CORE),
                )

        # ---- attention ----
        recip_dram = dpool.tile([NST, 4, 512], F32)

        def emit_attn(s):
            n_k = 4 * s + 4  # k-tiles for this strip (causal)
            yu_all = ppool.tile([HD + 1, HEADS_PER_CORE, 512], F32, name=f"yu_{s}", tag="yu", bufs=2)
            for p in range(PAIRS):
                ytile = [
                    y_ps.tile([HD + 1, 512], F32, name=f"y_{p}_{s}_{h}", tag=f"y{h}")
                    for h in range(2)
                ]
                ngrp = n_k // 2
                for g in range(ngrp):
                    st = [
                        st_ps.tile([128, 1024], F32, name=f"st_{p}_{s}_{g}_{h}", tag="st", bufs=2)
                        for h in range(2)
                    ]
                    # S^T matmuls, interleaved across heads for row-group overlap
                    for jj in range(2):
                        j = 2 * g + jj
                        c0 = max(0, 128 * (j - 4 * s))
                        for h in range(2):
                            nc.tensor.matmul(
                                st[h][:, jj * 512 + c0:(jj + 1) * 512],
                                kt[h * HD:(h + 1) * HD, p, j * 128:(j + 1) * 128],
                                qt[h * HD:(h + 1) * HD, p, s * 512 + c0:(s + 1) * 512],
                                start=True,
                                stop=True,
                            )
                    # exp, cropped to the first valid column of the group
                    ec0 = max(0, 128 * (2 * g - 4 * s))
                    for h in range(2):
                        hh = 2 * p + h
                        ptile = ppool.tile([128, 1024], BF16, name=f"pt_{p}_{s}_{g}_{h}", tag="pt", bufs=5)
                        nc.scalar.activation(
                            ptile[:, ec0:1024], st[h][:, ec0:1024],
                            mybir.ActivationFunctionType.Exp,
                        )
                        for jj in range(2):
                            j = 2 * g + jj
                            c0 = max(0, 128 * (j - 4 * s))
                            if j >= 4 * s:  # diagonal block: triangular mask
                                blk = ptile[:, jj * 512 + c0:jj * 512 + c0 + 128]
                                nc.gpsimd.tensor_mul(blk, blk, mask_tri[:])
                            nc.tensor.matmul(
                                ytile[h][:, c0:512],
                                v_nat[:, j, hh, :],
                                ptile[:, jj * 512 + c0:(jj + 1) * 512],
                                start=(j == 0),
                                stop=(j == n_k - 1),
                            )
                # evict y + sums rows to SBUF, freeing the PSUM banks
                for h in range(2):
                    hh = 2 * p + h
                    nc.vector.tensor_copy(yu_all[:, hh, :], ytile[h][:])

                # ---- per-pair normalization ----
                # SBUF->SBUF DMA moves the sums row (partition 64) to
                # partitions 0-1 (custom DVE ops only work at base 0), then
                # reciprocal, one DRAM hop for the partition-broadcast, and
                # Pool multiplies (all-SBUF operands).
                sums2 = ppool.tile([2, 512], F32, name=f"sums_{s}_{p}", tag=f"sums{p}", bufs=2)
                recip2 = ppool.tile([2, 512], F32, name=f"recip_{s}_{p}", tag=f"recip{p}", bufs=2)
                rscr2 = ppool.tile([2, 512], F32, name=f"rscr_{s}_{p}", tag=f"rscr{p}", bufs=2)
                rb2 = ppool.tile([HD, 2, 512], F32, name=f"rb_{s}_{p}", tag=f"rb{p}", bufs=2)
                nc.sync.dma_start(
                    sums2[:], yu_all[HD:HD + 1, 2 * p:2 * p + 2, :]
                )
                nc.vector.reciprocal_approx_accurate(recip2[:], sums2[:], rscr2[:])
                nc.sync.dma_start(recip_dram[s, 2 * p:2 * p + 2], recip2[:])
                nc.sync.dma_start(
                    rb2[:],
                    recip_dram[s:s + 1, 2 * p:2 * p + 2, :].to_broadcast((HD, 2, 512)),
                )
                for h in range(2):
                    r = 2 * p + h
                    nc.gpsimd.tensor_mul(
                        ytn[h * HD:(h + 1) * HD, p, s * 512:(s + 1) * 512],
                        yu_all[0:HD, r, :],
                        rb2[:, h, :],
                    )

        # ---- output projection (partial), interleaved per strip ----
        def emit_proj(s):
            for ti in range(4):
                t = 4 * s + ti
                ot = epool.tile([128, 1024], F32, name=f"ot_{t}", tag="ot", bufs=3)
                for n in range(2):
                    op = qkv_ps.tile([128, 512], F32, name=f"op_{t}_{n}", tag=f"qkv{n}")
                    for f in range(2):
                        nc.tensor.matmul(
                            op[:],
                            ytn[:, f, t * 128:(t + 1) * 128],
                            wp[:, f, n * 512:(n + 1) * 512],
                            start=(f == 0),
                            stop=(f == 1),
                        )
                    nc.vector.tensor_copy(ot[:, n * 512:(n + 1) * 512], op[:])
                nc.sync.dma_start(out_d[t * 128:(t + 1) * 128, :], ot[:])

        # ---- skewed software pipeline ----
        # per slot: attention first (highest priority), next strip's QKV and
        # the previous strip's projection as PE stall-filler
        emit_qkv(0, xs0)
        for s in range(NST):
            emit_attn(s)
            if s + 1 < NST:
                emit_qkv(s + 1, None)
            if s >= 1:
                emit_proj(s - 1)
        emit_proj(NST - 1)

    nc.compile()
    return nc


def _prep_inputs(x, W_attn, b_attn, W_proj):
    """Per-core input maps. Core k: batch k//4, head-group k%4."""
    assert np.allclose(b_attn, 0.0), "nonzero b_attn not supported by this kernel"
    scale = 1.0 / np.sqrt(np.float32(HD))

    mask = (np.arange(128)[:, None] <= np.arange(128)[None, :]).astype(NP_BF16)
    ones = np.ones((128, 1), dtype=NP_BF16)

    def lhsT_tiles(w):
        # [C, 128] -> [128, CKT, 128] with [p, t, c] = w[t*128+p, c]
        return np.ascontiguousarray(w.reshape(CKT, 128, 128).transpose(1, 0, 2))

    in_maps = []
    for core in range(NCORES):
        b = core // 4
        g = core % 4
        heads = [4 * g + i for i in range(HEADS_PER_CORE)]
        # [128, CKT, T] with [p, kc, t] = x^T[kc*128+p, t]
        xt = np.ascontiguousarray(
            x[b].T.reshape(CKT, 128, T).transpose(1, 0, 2)
        ).astype(NP_BF16)

        def w_slice(base, hs, sc=1.0):
            cols = np.concatenate(
                [np.arange(base + h * HD, base + (h + 1) * HD) for h in hs]
            )
            return np.ascontiguousarray(W_attn[:, cols]) * sc

        wq = np.stack(
            [lhsT_tiles(w_slice(0, heads[2 * p:2 * p + 2], scale)) for p in range(PAIRS)], axis=1
        ).astype(NP_BF16)  # [128, PAIRS, CKT, 128]
        wk = np.stack(
            [lhsT_tiles(w_slice(C, heads[2 * p:2 * p + 2])) for p in range(PAIRS)], axis=1
        ).astype(NP_BF16)
        # v weights in natural rhs layout: [128, CKT, 256], [p, kc, d] = Wv[kc*128+p, d]
        wv_cols = w_slice(2 * C, heads)  # [C, 256]
        wv = np.ascontiguousarray(
            wv_cols.reshape(CKT, 128, 256).transpose(1, 0, 2)
        ).astype(NP_BF16)
        # W_proj rows for this head group: [256, C] -> [128, 2, C]
        wp_rows = W_proj[heads[0] * HD:(heads[-1] + 1) * HD, :]
        wp = np.ascontiguousarray(wp_rows.reshape(2, 128, C).transpose(1, 0, 2))

        in_maps.append(
            {
                "xt": np.ascontiguousarray(xt),
                "wq": np.ascontiguousarray(wq),
                "wk": np.ascontiguousarray(wk),
                "wv": np.ascontiguousarray(wv),
                "wp": np.ascontiguousarray(wp, dtype=np.float32),
                "mask": np.ascontiguousarray(mask),
                "ones": ones,
            }
        )
    return in_maps


def kernel(x, W_attn, b_attn, W_proj, b_proj, _want_results=False, _spmd_kwargs=None):
    x = np.asarray(x, dtype=np.float32)
    W_attn = np.asarray(W_attn, dtype=np.float32)
    b_attn = np.asarray(b_attn, dtype=np.float32)
    W_proj = np.asarray(W_proj, dtype=np.float32)
    b_proj = np.asarray(b_proj, dtype=np.float32)

    if "nc" not in _CACHE:
        _CACHE["nc"] = _build()
    nc = _CACHE["nc"]

    in_maps = _prep_inputs(x, W_attn, b_attn, W_proj)
    kw = dict(_spmd_kwargs or {})
    res = run_bass_kernel_spmd(nc, in_maps, list(range(NCORES)), **kw)

    out = np.zeros((B, T, C), dtype=np.float32)
    for core in range(NCORES):
        out[core // 4] += np.asarray(res.results[core]["out"], dtype=np.float32)
    out += b_proj[None, None, :]
    if _want_results:
        return out, res
    return out


# revision 44
# speedup vs baseline: 1.1762x; 1.0041x over previous
"""Causal self-attention on 8 Trainium2 NeuronCores (Bass/Tile).

Problem shape (hardcoded): x [2, 2048, 1024], W_attn [1024, 3072],
b_attn [3072], W_proj [1024, 1024], b_proj [1024], 16 heads, hd=64.

Sharding: tensor-parallel over (batch, head-group). Core k handles
batch k//4 and heads 4*(k%4) .. 4*(k%4)+3 (two head-pairs). Each core
computes its 4 heads' attention and a partial output projection
(y_local @ W_proj[rows]) of shape [2048, 1024]; the host sums the four
partials per batch and adds b_proj.

v3: bf16 activations/weights (fp32 PSUM accumulation), V produced
directly in natural [t, d] layout (no PE transposes), Q/K PSUM
evictions on the scalar engine, per-pair softmax normalization with
the reciprocal read straight out of the y-eviction tile's sums row,
batched per-strip x DMAs, and attention emitted at higher priority
than the next strip's QKV and the previous strip's projection so the
tensor engine fills its softmax stalls with those matmuls.
"""

import sys

for _p in ("/opt/trn_rl_repo", "/root/.axon_site/_ro/trn_rl_repo"):
    if _p not in sys.path:
        sys.path.insert(0, _p)

import ml_dtypes
import numpy as np

import concourse.bass as bass  # noqa: F401  (engine types)
import concourse.mybir as mybir
import concourse.tile as tile
from concourse import bacc
from concourse.bass_utils import run_bass_kernel_spmd

F32 = mybir.dt.float32
F32R = mybir.dt.float32r
BF16 = mybir.dt.bfloat16
NP_BF16 = ml_dtypes.bfloat16

B = 2
T = 2048
C = 1024
H = 16
HD = 64
NCORES = 8
HEADS_PER_CORE = 4  # two pairs
PAIRS = 2
NKT = T // 128       # 16 k-tiles per head
NST = T // 512       # 4 q-strips per head
CKT = C // 128       # 8 contraction tiles for C

_CACHE = {}


def _build():
    """Build the SPMD Bass program (identical for all cores)."""
    nc = bacc.Bacc(None, target_bir_lowering=False)

    # x^T pre-tiled on host: [p, kc, t] = x^T[kc*128+p, t]
    xt_d = nc.dram_tensor("xt", [128, CKT, T], BF16, kind="ExternalInput")
    wq_d = nc.dram_tensor("wq", [128, PAIRS, CKT, 128], BF16, kind="ExternalInput")
    wk_d = nc.dram_tensor("wk", [128, PAIRS, CKT, 128], BF16, kind="ExternalInput")
    wv_d = nc.dram_tensor("wv", [128, CKT, 256], BF16, kind="ExternalInput")
    wp_d = nc.dram_tensor("wp", [128, 2, C], F32R, kind="ExternalInput")
    mask_d = nc.dram_tensor("mask", [128, 128], BF16, kind="ExternalInput")
    ones_d = nc.dram_tensor("ones", [128, 1], BF16, kind="ExternalInput")
    out_d = nc.dram_tensor("out", [T, C], BF16, kind="ExternalOutput")

    with tile.TileContext(nc) as tc, (
        tc.tile_pool(name="const", bufs=1)
    ) as const, (
        tc.tile_pool(name="weights", bufs=1)
    ) as wpool, (
        tc.tile_pool(name="acts", bufs=1)
    ) as apool, (
        tc.tile_pool(name="xstream", bufs=3)
    ) as xpool, (
        tc.tile_pool(name="ptp", bufs=3)
    ) as ppool, (
        tc.tile_pool(name="evict", bufs=3)
    ) as epool, (
        tc.tile_pool(name="dram_bounce", bufs=1, space="DRAM")
    ) as dpool, (
        tc.tile_pool(name="st_ps", bufs=1, space="PSUM")
    ) as st_ps, (
        tc.tile_pool(name="y_ps", bufs=1, space="PSUM")
    ) as y_ps, (
        tc.tile_pool(name="qkv_ps", bufs=1, space="PSUM")
    ) as qkv_ps:
        mask_tri = const.tile([128, 128], BF16)

        wq = wpool.tile([128, PAIRS, CKT, 128], BF16)
        wk = wpool.tile([128, PAIRS, CKT, 128], BF16)
        wv = wpool.tile([128, CKT, 256], BF16)
        wp = wpool.tile([128, 2, C], F32R)

        # activations kept resident in SBUF
        qt = apool.tile([128, PAIRS, T], BF16)   # q^T, heads stacked in pairs
        kt = apool.tile([128, PAIRS, T], BF16)   # k^T
        v_nat = apool.tile([128, NKT, HEADS_PER_CORE, HD + 1], BF16)
        ytn = apool.tile([128, PAIRS, T], F32R)  # normalized y^T

        # ---- lead-in DMAs: first strip of x + pair-0 weights first ----
        xs0 = xpool.tile([128, CKT, 512], BF16, name="xc_0", tag="xc")
        nc.sync.dma_start(xs0[:, 0:4], xt_d[:, 0:4, 0:512])
        nc.sync.dma_start(wq[:, 0], wq_d[:, 0])
        nc.sync.dma_start(xs0[:, 4:8], xt_d[:, 4:8, 0:512])
        nc.sync.dma_start(wk[:, 0], wk_d[:, 0])
        nc.sync.dma_start(wv[:], wv_d[:])
        nc.sync.dma_start(mask_tri[:], mask_d[:])
        nc.sync.dma_start(wq[:, 1], wq_d[:, 1])
        nc.sync.dma_start(wk[:, 1], wk_d[:, 1])
        nc.sync.dma_start(wp[:], wp_d[:])
        # ones column of v_nat (the 65th lhsT column yields softmax sums)
        for hh in range(HEADS_PER_CORE):
            nc.sync.dma_start(
                v_nat[:, :, hh, HD:HD + 1], ones_d[:].to_broadcast((128, NKT, 1))
            )

        # ---- QKV (q^T/k^T transposed; v natural) ----
        def emit_qkv(s, xs):
            if xs is None:
                xs = xpool.tile([128, CKT, 512], BF16, name=f"xc_{s}", tag="xc")
                nc.sync.dma_start(xs[:], xt_d[:, :, s * 512:(s + 1) * 512])
            for p in range(PAIRS):
                for w_t, dest in ((wq, qt), (wk, kt)):
                    ps = qkv_ps.tile([128, 512], F32, name=f"qkps_{s}_{p}_{0 if w_t is wq else 1}", tag=f"qkv{p}")
                    for kc in range(CKT):
                        nc.tensor.matmul(
                            ps[:],
                            w_t[:, p, kc, :],
                            xs[:, kc, :],
                            start=(kc == 0),
                            stop=(kc == CKT - 1),
                        )
                    # evict on the scalar engine (Act is idle during QKV)
                    nc.scalar.copy(dest[:, p, s * 512:(s + 1) * 512], ps[:])
            # v in natural layout: x^T chunks as lhsT
            for i in range(4):
                t = 4 * s + i
                psv = qkv_ps.tile([128, 256], F32, name=f"vps_{s}_{i}", tag=f"qkv{i % 2}")
                for kc in range(CKT):
                    nc.tensor.matmul(
                        psv[:],
                        xs[:, kc, i * 128:(i + 1) * 128],
                        wv[:, kc, :],
                        start=(kc == 0),
                        stop=(kc == CKT - 1),
                    )
                nc.vector.tensor_copy(
                    v_nat[:, t, :, 0:HD],
                    psv[:].rearrange("p (h d) -> p h d", h=HEADS_PER_CORE),
                )

        # ---- attention ----
        recip_dram = dpool.tile([NST, 4, 512], F32)

        def emit_attn(s):
            n_k = 4 * s + 4  # k-tiles for this strip (causal)
            yu_all = ppool.tile([HD + 1, HEADS_PER_CORE, 512], F32, name=f"yu_{s}", tag="yu", bufs=2)
            for p in range(PAIRS):
                ytile = [
                    y_ps.tile([HD + 1, 512], F32, name=f"y_{p}_{s}_{h}", tag=f"y{h}")
                    for h in range(2)
                ]
                ngrp = n_k // 2
                for g in range(ngrp):
                    st = [
                        st_ps.tile([128, 1024], F32, name=f"st_{p}_{s}_{g}_{h}", tag="st", bufs=2)
                        for h in range(2)
                    ]
                    # S^T matmuls, interleaved across heads for row-group overlap
                    for jj in range(2):
                        j = 2 * g + jj
                        c0 = max(0, 128 * (j - 4 * s))
                        for h in range(2):
                            nc.tensor.matmul(
                                st[h][:, jj * 512 + c0:(jj + 1) * 512],
                                kt[h * HD:(h + 1) * HD, p, j * 128:(j + 1) * 128],
                                qt[h * HD:(h + 1) * HD, p, s * 512 + c0:(s + 1) * 512],
                                start=True,
                                stop=True,
                            )
                    # exp, cropped to the first valid column of the group
                    ec0 = max(0, 128 * (2 * g - 4 * s))
                    for h in range(2):
                        hh = 2 * p + h
                        ptile = ppool.tile([128, 1024], BF16, name=f"pt_{p}_{s}_{g}_{h}", tag="pt", bufs=5)
                        nc.scalar.activation(
                            ptile[:, ec0:1024], st[h][:, ec0:1024],
                            mybir.ActivationFunctionType.Exp,
                        )
                        for jj in range(2):
                            j = 2 * g + jj
                            c0 = max(0, 128 * (j - 4 * s))
                            if j >= 4 * s:  # diagonal block: triangular mask
                                blk = ptile[:, jj * 512 + c0:jj * 512 + c0 + 128]
                                nc.gpsimd.tensor_mul(blk, blk, mask_tri[:])
                            nc.tensor.matmul(
                                ytile[h][:, c0:512],
                                v_nat[:, j, hh, :],
                                ptile[:, jj * 512 + c0:(jj + 1) * 512],
                                start=(j == 0),
                                stop=(j == n_k - 1),
                            )
                # evict y + sums rows to SBUF, freeing the PSUM banks
                for h in range(2):
                    hh = 2 * p + h
                    nc.vector.tensor_copy(yu_all[:, hh, :], ytile[h][:])

                # ---- per-pair normalization ----
                # SBUF->SBUF DMA moves the sums row (partition 64) to
                # partition base 0 (custom DVE ops only work at base 0).
                # (custom DVE ops only work at base 0), then reciprocal,
                # one DRAM hop for the partition-broadcast, Pool muls.
                sums2 = ppool.tile([2, 512], F32, name=f"sums_{s}_{p}", tag=f"sums{p}", bufs=2)
                recip2 = ppool.tile([2, 512], F32, name=f"recip_{s}_{p}", tag=f"recip{p}", bufs=2)
                rscr2 = ppool.tile([2, 512], F32, name=f"rscr_{s}_{p}", tag=f"rscr{p}", bufs=2)
                rb2 = ppool.tile([HD, 2, 512], F32, name=f"rb_{s}_{p}", tag=f"rb{p}", bufs=2)
                nc.sync.dma_start(
                    sums2[:], yu_all[HD:HD + 1, 2 * p:2 * p + 2, :]
                )
                nc.vector.reciprocal_approx_accurate(recip2[:], sums2[:], rscr2[:])
                nc.sync.dma_start(recip_dram[s, 2 * p:2 * p + 2], recip2[:])
                nc.sync.dma_start(
                    rb2[:],
                    recip_dram[s:s + 1, 2 * p:2 * p + 2, :].to_broadcast((HD, 2, 512)),
                )
                for h in range(2):
                    r = 2 * p + h
                    nc.gpsimd.tensor_mul(
                        ytn[h * HD:(h + 1) * HD, p, s * 512:(s + 1) * 512],
                        yu_all[0:HD, r, :],
                        rb2[:, h, :],
                    )

        # ---- output projection (partial), interleaved per strip ----
        def emit_proj(s):
            for ti in range(4):
                t = 4 * s + ti
                ot = epool.tile([128, 1024], BF16, name=f"ot_{t}", tag="ot", bufs=3)
                for n in range(2):
                    op = qkv_ps.tile([128, 512], F32, name=f"op_{t}_{n}", tag=f"qkv{n}")
                    for f in range(2):
                        nc.tensor.matmul(
                            op[:],
                            ytn[:, f, t * 128:(t + 1) * 128],
                            wp[:, f, n * 512:(n + 1) * 512],
                            start=(f == 0),
                            stop=(f == 1),
                        )
                    nc.vector.tensor_copy(ot[:, n * 512:(n + 1) * 512], op[:])
                nc.sync.dma_start(out_d[t * 128:(t + 1) * 128, :], ot[:])

        # ---- skewed software pipeline ----
        # per slot: attention first (highest priority), next strip's QKV and
        # the previous strip's projection as PE stall-filler
        emit_qkv(0, xs0)
        for s in range(NST):
            emit_attn(s)
            if s + 1 < NST:
                emit_qkv(s + 1, None)
            if s >= 1:
                emit_proj(s - 1)
        emit_proj(NST - 1)

    nc.compile()
    return nc


def _prep_inputs(x, W_attn, b_attn, W_proj):
    """Per-core input maps. Core k: batch k//4, head-group k%4."""
    assert np.allclose(b_attn, 0.0), "nonzero b_attn not supported by this kernel"
    scale = 1.0 / np.sqrt(np.float32(HD))

    mask = (np.arange(128)[:, None] <= np.arange(128)[None, :]).astype(NP_BF16)
    ones = np.ones((128, 1), dtype=NP_BF16)

    def lhsT_tiles(w):
        # [C, 128] -> [128, CKT, 128] with [p, t, c] = w[t*128+p, c]
        return np.ascontiguousarray(w.reshape(CKT, 128, 128).transpose(1, 0, 2))

    in_maps = []
    for core in range(NCORES):
        b = core // 4
        g = core % 4
        heads = [4 * g + i for i in range(HEADS_PER_CORE)]
        # [128, CKT, T] with [p, kc, t] = x^T[kc*128+p, t]
        xt = np.ascontiguousarray(
            x[b].T.reshape(CKT, 128, T).transpose(1, 0, 2)
        ).astype(NP_BF16)

        def w_slice(base, hs, sc=1.0):
            cols = np.concatenate(
                [np.arange(base + h * HD, base + (h + 1) * HD) for h in hs]
            )
            return np.ascontiguousarray(W_attn[:, cols]) * sc

        wq = np.stack(
            [lhsT_tiles(w_slice(0, heads[2 * p:2 * p + 2], scale)) for p in range(PAIRS)], axis=1
        ).astype(NP_BF16)  # [128, PAIRS, CKT, 128]
        wk = np.stack(
            [lhsT_tiles(w_slice(C, heads[2 * p:2 * p + 2])) for p in range(PAIRS)], axis=1
        ).astype(NP_BF16)
        # v weights in natural rhs layout: [128, CKT, 256], [p, kc, d] = Wv[kc*128+p, d]
        wv_cols = w_slice(2 * C, heads)  # [C, 256]
        wv = np.ascontiguousarray(
            wv_cols.reshape(CKT, 128, 256).transpose(1, 0, 2)
        ).astype(NP_BF16)
        # W_proj rows for this head group: [256, C] -> [128, 2, C]
        wp_rows = W_proj[heads[0] * HD:(heads[-1] + 1) * HD, :]
        wp = np.ascontiguousarray(wp_rows.reshape(2, 128, C).transpose(1, 0, 2))

        in_maps.append(
            {
                "xt": np.ascontiguousarray(xt),
                "wq": np.ascontiguousarray(wq),
                "wk": np.ascontiguousarray(wk),
                "wv": np.ascontiguousarray(wv),
                "wp": np.ascontiguousarray(wp, dtype=np.float32),
                "mask": np.ascontiguousarray(mask),
                "ones": ones,
            }
        )
    return in_maps


def kernel(x, W_attn, b_attn, W_proj, b_proj, _want_results=False, _spmd_kwargs=None):
    x = np.asarray(x, dtype=np.float32)
    W_attn = np.asarray(W_attn, dtype=np.float32)
    b_attn = np.asarray(b_attn, dtype=np.float32)
    W_proj = np.asarray(W_proj, dtype=np.float32)
    b_proj = np.asarray(b_proj, dtype=np.float32)

    if "nc" not in _CACHE:
        _CACHE["nc"] = _build()
    nc = _CACHE["nc"]

    in_maps = _prep_inputs(x, W_attn, b_attn, W_proj)
    kw = dict(_spmd_kwargs or {})
    res = run_bass_kernel_spmd(nc, in_maps, list(range(NCORES)), **kw)

    out = np.zeros((B, T, C), dtype=np.float32)
    for core in range(NCORES):
        out[core // 4] += np.asarray(res.results[core]["out"], dtype=np.float32)
    out += b_proj[None, None, :]
    if _want_results:
        return out, res
    return out
